# revision 27
# baseline (speedup 1.0000x reference)
"""Trainium2 Bass kernel for nn_BatchBayesianLogicCell.

Shapes (hardcoded): P=Q=64 predicates/questions, A=2 arity, O=1024 objects,
batch_object_map is block-diagonal with G = O//Q = 16 objects per question,
dim_order = [0, 1].

Math reduction
--------------
The reference computes, per branch a in {0,1} (with dims=[0,1]):
  t    = pnot(ll + prior_j (broadcast along obj-dim j), alpha_j)   [P,O,O]
  t[diag] = 0
  pool = einsum over obj-dim j with bmap -> question axis           [P,*,Q]
  u    = pnot(pool, alpha_j) + prior_i (broadcast along obj-dim i)
  res  = (u * bmap^T).sum(question axis)                            [P,O]
Because bmap is block-diagonal AND the final masked sum selects, for each
object n, exactly the question q(n) = n // 16 that owns it, only the pooled
value at q = q(n) survives.  That pooled value sums t over the 16 objects in
n's own group.  Hence only the 64 diagonal 16x16 blocks of ll (per predicate)
ever matter: 4 MB of the 256 MB input.

Device layout (per core, 8 predicates):
  partition = (local_pred, within-block index) -> 8*16 = 128 partitions
  free      = branch-concat of [64 groups x 16 block-col] = 2048
  x[:, :1024]  branch0: block-rows on partitions, prior1 pre-added (host)
  x[:, 1024:]  branch1: block-cols on partitions, prior0 pre-added (host)

Diagonal zeroing: every in-block diagonal entry is poisoned to -88 on the
host.  exp(-88) == 0 in fp32, so log(1-exp(-88)) == 0 and the alpha=1 linear
weight (1-alpha)=0 kills the -88: the diagonal contributes exactly 0 to the
alpha=1 sums.  For alpha=0 rows the diagonal contributes a spurious -88 to
the segment sum; that is compensated exactly by feeding the second exp a
per-partition bias of 88*(1-alpha) and baking +88*(1-alpha) into the prior
grid (both only act on alpha=0 rows, where the correction is exact since
alpha is binary).  This removes every clamp from the device pipeline and no
intermediate is ever nonfinite.

Device pipeline (per branch b, alpha_b per-partition, all fp32):
  e   = Exp(x)                                   [ACT]
  lg  = Ln(-e + 1)          (fused scale/bias)   [ACT]
  t   = (lg * alpha_b) + (x * (1-alpha_b))       [DVE ts + stt]
  s   = segment_sum_16(t)                        [DVE reduce]
  e2  = Exp(s + fix_b)      (per-partition bias) [ACT]
  lg2 = Ln(-e2 + 1)                              [ACT]
  res = (lg2 * alpha_b) + ((s * (1-alpha_b)) + pgrid_adj)   [DVE stt x2]
"""

import numpy as np

P, A, O, Q = 64, 2, 1024, 64
G = O // Q            # 16 objects per question group
NCORES = 8
PPC = P // NCORES     # 8 predicates per core
POISON = np.float32(-88.0)  # exp(-88) == 0 in fp32 -> log(1-exp) == 0

TRACE = False          # set True (e.g. from test.py) to collect an NTFF profile
LAST_RESULT = None     # BassKernelResults of the last device run

H = Q * G              # 1024, one branch's free extent
NA = 8                 # alpha-section columns (duplicated per DMA chunk)
# Uneven pipeline chunks (2 per branch): a big head chunk and a small tail
# chunk so the last result leaves for DRAM as early as possible (the output
# DMA completion + end barrier dominate the kernel tail).
SZ = [768, 256, 768, 256]
NCHUNK = len(SZ)
COFF = [sum(SZ[:c]) for c in range(NCHUNK + 1)]  # x-offsets, COFF[-1] = 2048
W = 2 * H + NCHUNK * NA  # SBUF packed width (x chunks + per-chunk alpha dups)


def _patched_act_tables(orig):
    """Steer the act-table chooser to the one table that has BOTH Exp and Ln
    (natural_log_exp_and_others) so the kernel needs a single table load
    instead of swapping Exp/Ln tables four times.  Order (and therefore
    act_func_set_id numbering) is preserved."""
    import concourse.mybir as mybir

    drop = {mybir.ActivationFunctionType.Exp, mybir.ActivationFunctionType.Ln}

    def patched(arch):
        tabs = orig(arch)
        return {
            name: (s if name == "natural_log_exp_and_others" else s - drop)
            for name, s in tabs.items()
        }

    return patched


def _build_nc():
    import concourse.mybir as mybir
    import concourse.tile as tile
    from concourse import bacc

    f32 = mybir.dt.float32
    Exp = mybir.ActivationFunctionType.Exp
    Ln = mybir.ActivationFunctionType.Ln
    AX = mybir.AxisListType.X
    MUL = mybir.AluOpType.mult
    ADD = mybir.AluOpType.add

    nc = bacc.Bacc("TRN2", target_bir_lowering=False, debug=False)
    xins = [
        nc.dram_tensor(f"xin{c}", [128, SZ[c] + NA], f32, kind="ExternalInput")
        for c in range(NCHUNK)
    ]
    xpg = nc.dram_tensor("xpg", [128, 2 * Q], f32, kind="ExternalInput")
    res = nc.dram_tensor("res", [128, 2 * Q], f32, kind="ExternalOutput")

    with tile.TileContext(nc) as tc:
        with tc.tile_pool(name="pool", bufs=1) as pool:
            # Four parallel input DMAs on separate queues, triggered from
            # otherwise-idle engines so the trigger instructions don't
            # serialize.  Alpha columns are duplicated into every chunk so no
            # consumer instruction ever needs to wait on more than one DMA
            # semaphore (HW encodes at most one wait per instruction).
            xa = pool.tile([128, W], f32)
            trig = [nc.scalar, nc.gpsimd, nc.sync, nc.sync]
            for c in range(NCHUNK):
                off = COFF[c] + c * NA
                trig[c].dma_start(xa[:, off : off + SZ[c] + NA], xins[c][:])
            pgt = pool.tile([128, 2 * Q], f32)
            nc.gpsimd.dma_start(pgt[:], xpg[:])

            def xc(c):
                off = COFF[c] + c * NA
                return xa[:, off : off + SZ[c]]

            def ac(c, col):
                off = COFF[c] + c * NA + SZ[c] + col
                return xa[:, off : off + 1]

            pg = pgt[:]

            # pnot big stage, chunked: t = alpha*log(1-exp(x)) + (1-alpha)*x
            # chunk c belongs to branch c // 2 (alpha cols 0/1 vs 2/3)
            e = pool.tile([128, 2 * H], f32)
            lg = pool.tile([128, 2 * H], f32)
            t2 = pool.tile([128, 2 * H], f32)
            t = pool.tile([128, 2 * H], f32)
            s = pool.tile([128, 2 * Q], f32)
            for c in range(NCHUNK):
                b = c // 2
                sl = slice(COFF[c], COFF[c + 1])
                nc.scalar.activation(e[:, sl], xc(c), Exp)
                nc.scalar.activation(lg[:, sl], e[:, sl], Ln, bias=1.0, scale=-1.0)
                nc.vector.tensor_scalar_mul(t2[:, sl], xc(c), ac(c, 2 * b + 1))
                nc.vector.scalar_tensor_tensor(
                    t[:, sl], lg[:, sl], ac(c, 2 * b), t2[:, sl], MUL, ADD
                )
                # segment-sum over the 16 in-block entries
                nc.vector.reduce_sum(
                    s[:, COFF[c] // G : COFF[c + 1] // G],
                    t[:, sl].rearrange("p (g c) -> p g c", c=G),
                    axis=AX,
                )

            # pnot on the pooled tile (+88 bias un-poisons alpha=0 rows),
            # split per branch so each half finishes as soon as its two
            # reduce chunks are done.
            e2 = pool.tile([128, 2 * Q], f32)
            lg2 = pool.tile([128, 2 * Q], f32)
            w = pool.tile([128, 2 * Q], f32)
            r = pool.tile([128, 2 * Q], f32)
            for b in range(2):
                sb = slice(b * Q, (b + 1) * Q)
                cb = 2 * b + 1  # chunk whose alpha copy we read
                nc.scalar.activation(e2[:, sb], s[:, sb], Exp, bias=ac(cb, 4 + b))
                nc.scalar.activation(lg2[:, sb], e2[:, sb], Ln, bias=1.0, scale=-1.0)
                nc.vector.scalar_tensor_tensor(
                    w[:, sb], s[:, sb], ac(cb, 2 * b + 1), pg[:, sb], MUL, ADD
                )
                nc.vector.scalar_tensor_tensor(
                    r[:, sb], lg2[:, sb], ac(cb, 2 * b), w[:, sb], MUL, ADD
                )
                nc.sync.dma_start(res[:, sb], r[:, sb])

    orig_gat = bacc.get_activation_tables
    bacc.get_activation_tables = _patched_act_tables(orig_gat)
    try:
        nc.finalize()
    finally:
        bacc.get_activation_tables = orig_gat
    return nc


def _prep_inputs(log_prior, ll, quant):
    """Host-side shard/layout prep. Returns in_maps for the 8 cores."""
    prior0 = log_prior[:, 0, :]  # [P, O]
    prior1 = log_prior[:, 1, :]

    # Extract the diagonal 16x16 blocks: blk[p, q, r, c] = ll[p, 16q+r, 16q+c]
    ll5 = ll.reshape(P, Q, G, Q, G)
    qi = np.arange(Q)
    blk = ll5[:, qi, :, qi, :]          # [Q, P, G, G] (advanced idx dims first)
    blk = np.minimum(blk, 0.0).transpose(1, 0, 2, 3).astype(np.float32)  # [P,Q,G,G]

    # Pre-add the prior broadcast (matches reference op order: min -> +prior)
    a0 = blk + prior1.reshape(P, Q, 1, G)  # branch0: + prior1[p, 16q+c]
    a1 = blk + prior0.reshape(P, Q, G, 1)  # branch1: + prior0[p, 16q+r]

    # Poison the in-block diagonal (see module docstring)
    ii = np.arange(G)
    a0[:, :, ii, ii] = POISON
    a1[:, :, ii, ii] = POISON

    ab0 = quant[:, 1].astype(np.float32)  # alpha for branch a=0 (j=2)
    ab1 = quant[:, 0].astype(np.float32)  # alpha for branch a=1 (j=1)
    omab0 = (np.float32(1.0) - ab0).astype(np.float32)
    omab1 = (np.float32(1.0) - ab1).astype(np.float32)
    fix0 = (-POISON) * omab0  # +88 on alpha=0 rows, 0 on alpha=1 rows
    fix1 = (-POISON) * omab1

    # Device layouts: branch0 partitions = block-row r, branch1 partitions = block-col c
    x0 = a0.transpose(0, 2, 1, 3).reshape(P, G, Q * G)  # [P, r, (q,c)]
    x1 = a1.transpose(0, 3, 1, 2).reshape(P, G, Q * G)  # [P, c, (q,r)]

    acols = np.stack(
        [ab0, omab0, ab1, omab1, fix0, fix1, np.zeros(P, np.float32), np.zeros(P, np.float32)],
        axis=1,
    ).astype(np.float32)  # [P, 8]

    # prior grid for the output stage, with the alpha=0 poison compensation
    pg0 = prior0.reshape(P, Q, G).transpose(0, 2, 1) + fix0[:, None, None]  # [P, r, q]
    pg1 = prior1.reshape(P, Q, G).transpose(0, 2, 1) + fix1[:, None, None]
    pgrid = np.concatenate([pg0, pg1], axis=2).astype(np.float32)  # [P, 16, 128]

    in_maps = []
    for c in range(NCORES):
        sl = slice(c * PPC, (c + 1) * PPC)
        ac_rep = np.repeat(acols[sl], G, axis=0)  # [128, 8]
        x0r = x0[sl].reshape(128, Q * G)
        x1r = x1[sl].reshape(128, Q * G)
        xparts = [
            x0r[:, 0 : SZ[0]],
            x0r[:, SZ[0] : H],
            x1r[:, 0 : SZ[2]],
            x1r[:, SZ[2] : H],
        ]
        m = {
            f"xin{k}": np.ascontiguousarray(
                np.concatenate([xparts[k], ac_rep], axis=1)
            )
            for k in range(NCHUNK)
        }
        m["xpg"] = np.ascontiguousarray(pgrid[sl].reshape(128, 2 * Q))
        in_maps.append(m)
    return in_maps


def _run_device(in_maps):
    global LAST_RESULT
    from concourse.bass_utils import run_bass_kernel_spmd

    nc = _build_nc()
    LAST_RESULT = run_bass_kernel_spmd(
        nc, in_maps, list(range(NCORES)), trace=TRACE
    )
    return LAST_RESULT.results


def _assemble(results):
    out = np.zeros((P, A, O), dtype=np.float32)
    for c in range(NCORES):
        r = np.asarray(results[c]["res"]).reshape(PPC, G, 2 * Q)
        res0 = r[:, :, 0:Q]      # [pl, r, q] -> out[pred, 0, 16q+r]
        res1 = r[:, :, Q : 2 * Q]
        sl = slice(c * PPC, (c + 1) * PPC)
        out[sl, 0, :] = res0.transpose(0, 2, 1).reshape(PPC, O)
        out[sl, 1, :] = res1.transpose(0, 2, 1).reshape(PPC, O)
    return out


# ---------------------------------------------------------------------------
# Fallback: faithful numpy port of the reference, used only if the inputs do
# not match the hardcoded structure (block-diagonal bmap, dims=[0,1], binary
# quantifiers).  Slow but correct for arbitrary inputs.
# ---------------------------------------------------------------------------

def _pnot_np(x, alpha):
    ex = np.exp(np.minimum(x, np.float32(0.0)))
    lg = np.log(np.clip(np.float32(1.0) - ex, np.float32(1e-12), None))
    return (alpha * lg + (np.float32(1.0) - alpha) * x).astype(np.float32)


def _reference_numpy(log_prior, ll4, quant, dims, bmap):
    ll = np.minimum(ll4.mean(axis=-1, dtype=np.float32), np.float32(0.0))
    diag = np.arange(O)
    out = np.zeros((P, A, O), dtype=np.float32)
    for a in range(2):
        i = dims[a] + 1
        j = dims[1 - a] + 1
        qj = quant[:, j - 1][:, None, None].astype(np.float32)
        if j == 1:
            lp = ll + log_prior[:, 0, :][:, :, None]
        else:
            lp = ll + log_prior[:, 1, :][:, None, :]
        lp = _pnot_np(lp, qj)
        lp[:, diag, diag] = 0.0
        if j == 1:
            lp = np.einsum("qo,pon->pqn", bmap, lp).astype(np.float32)
        else:
            lp = np.einsum("qo,pno->pnq", bmap, lp).astype(np.float32)
        lp = _pnot_np(lp, qj)
        if i == 1:
            lp = lp + log_prior[:, 0, :][:, :, None]
        else:
            lp = lp + log_prior[:, 1, :][:, None, :]
        if i == 2:
            lp = np.transpose(lp, (0, 2, 1))
        out[:, i - 1, :] = (lp * bmap.T[None, :, :]).sum(axis=2)
    return out


def kernel(log_prior, log_likelihood, quantifiers, dim_order, batch_object_map):
    log_prior = np.asarray(log_prior, dtype=np.float32)
    ll = np.asarray(log_likelihood, dtype=np.float32)
    quant = np.asarray(quantifiers, dtype=np.float32)
    dims = [int(v) for v in np.asarray(dim_order)]
    bmap = np.asarray(batch_object_map, dtype=np.float32)

    expected_bmap = (
        np.arange(O)[None, :] // G == np.arange(Q)[:, None]
    ).astype(np.float32)
    structured = (
        log_prior.shape == (P, A, O)
        and ll.shape == (P, O, O, 1)
        and quant.shape == (Q, A)
        and bmap.shape == (Q, O)
        and dims == [0, 1]
        and np.array_equal(bmap, expected_bmap)
        and bool(np.all((quant == 0.0) | (quant == 1.0)))
    )
    if not structured:
        return _reference_numpy(log_prior, ll, quant, dims, bmap)

    in_maps = _prep_inputs(log_prior, ll, quant)
    results = _run_device(in_maps)
    return _assemble(results)
